# revision 1
# baseline (speedup 1.0000x reference)
"""EnergyMACE Trainium2 kernel: edge/graph-parallel over 8 NeuronCores.

Sharding: nodes are partitioned by receiver range (2048/core), re-sorted by
species into 2560 padded slots (256/species) so species-dependent weights map
to fixed node-tiles. Edges are binned by the node-tile of their receiver and
padded to 18 subtiles of 128 edges per node-tile. The segment-sum is an
indicator one-hot matmul accumulated in PSUM per node-tile. Node features for
layer 2 are exchanged with one AllGather.
"""
import sys
import numpy as np

for p in ("/opt/trn_rl_repo", "/root/.axon_site/_ro/trn_rl_repo"):
    if p not in sys.path:
        sys.path.insert(0, p)

import ml_dtypes  # noqa: E402

N, E, S, K, NB = 16384, 262144, 10, 64, 8
R_MAX, AVG = 5.0, 16.0
NCORE = 8
NPC = N // NCORE          # 2048 real nodes / core
SPS = 256                 # padded slots per species
NSLOT = SPS * S           # 2560 slots / core
NT = NSLOT // 128         # 20 node tiles / core
ST = 18                   # edge subtiles (128 edges) per node tile
NSUB = NT * ST            # 360 subtiles / core
EPC = NSUB * 128          # 46080 padded edges / core
TBN = NCORE * NSLOT       # 20480 table rows
NGRP = NSUB // 8          # 45 geometry groups of 8 subtiles

S3 = float(np.sqrt(3.0, dtype=np.float32))
S15 = float(np.sqrt(15.0, dtype=np.float32))
S5 = float(np.sqrt(5.0, dtype=np.float32))
SQ25 = float(np.float32(np.sqrt(2.0 / R_MAX)))
LM = [0, 1, 1, 1, 2, 2, 2, 2, 2]

_prog_cache = {}


def _build_program(debug=False):
    if ("nc", debug) in _prog_cache:
        return _prog_cache[("nc", debug)]
    from contextlib import ExitStack
    from concourse import bass, bacc, mybir, tile
    from concourse.masks import make_identity

    f32 = mybir.dt.float32
    bf16 = mybir.dt.bfloat16
    i32 = mybir.dt.int32
    AF = mybir.ActivationFunctionType
    OP = mybir.AluOpType
    AX = mybir.AxisListType

    nc = bacc.Bacc("TRN2", target_bir_lowering=False, debug=False,
                   num_devices=NCORE)

    din = {}
    def inp(name, shape, dt):
        din[name] = nc.dram_tensor(name, shape, dt, kind="ExternalInput").ap()
    inp("pos_pad", [N, 64], f32)
    inp("W_embed", [S, 64], f32)
    inp("wrad_rep", [128, 2, 192], bf16)
    inp("wmix", [64, 2, 3, 64], f32)
    inp("wsc00", [64, S, 64], f32)
    inp("wsc10", [64, S, 64], f32)
    inp("wp_rep", [128, 2, 3, 64], f32)
    inp("wro0_rep", [128, 64], f32)
    inp("wm1", [64, 16], f32)
    inp("bm1_rep", [128, 16], f32)
    inp("wm2_rep", [128, 16], f32)
    inp("npi_rep", [128, NB], f32)
    inp("idx_spos", [128, NSUB], i32)
    inp("idx_rpos", [128, NSUB], i32)
    inp("idx_f1", [128, NSUB], i32)
    inp("idx_f2", [128, NSUB], i32)
    inp("idx_spec_all", [128, TBN // 128], i32)
    inp("recvloc", [128, NSUB], f32)
    out_e = nc.dram_tensor("out_e", [128, 2, NT], f32,
                           kind="ExternalOutput").ap()
    dbg = {}
    if debug:
        for nm, shp, dt in [("dbg_Y", [128, NSUB * 9], bf16),
                            ("dbg_rad", [8, ST * 128], bf16),
                            ("dbg_hs", [128, ST * 64], f32),
                            ("dbg_ind", [128, ST * 128], bf16),
                            ("dbg_msg", [128, 576], bf16),
                            ("dbg_A", [128, 576], f32),
                            ("dbg_Am", [128, 576], f32),
                            ("dbg_f0", [128, NT * 64], f32),
                            ("dbg_radg", [128, 64], f32),
                            ("dbg_sarg", [128, 64], f32),
                            ("dbg_sn", [128, 64], f32),
                            ("dbg_env", [128, 8], f32),
                            ("dbg_rinv", [128, 8], f32)]:
            dbg[nm] = nc.dram_tensor(nm, shp, dt, kind="ExternalOutput").ap()
    # internal DRAM (offset-0 tensors so indirect DMA can gather from them)
    T0 = nc.dram_tensor("T0", [TBN, 64], f32, kind="Internal").ap()
    T1s = nc.dram_tensor("T1s", [NSLOT, 64], f32, kind="Internal").ap()
    T1f = nc.dram_tensor("T1f", [TBN, 64], f32, kind="Internal",
                         addr_space="Shared").ap()
    radT_d = nc.dram_tensor("radT_d", [8, NSUB * 128], bf16,
                            kind="Internal").ap()

    with tile.TileContext(nc) as tc, ExitStack() as ctx:
        const = ctx.enter_context(tc.tile_pool(name="const", bufs=1))
        work = ctx.enter_context(tc.tile_pool(name="work", bufs=3))
        gwork = ctx.enter_context(tc.tile_pool(name="gwork", bufs=2))
        pers = ctx.enter_context(tc.tile_pool(name="pers", bufs=1))
        psum = ctx.enter_context(tc.tile_pool(name="psum", bufs=2, space="PSUM"))
        psA_p = ctx.enter_context(tc.tile_pool(name="psA", bufs=1, space="PSUM"))

        def load(name):
            src = din[name]
            t = const.tile(list(src.shape), src.dtype, tag=name)
            nc.sync.dma_start(t[:], src[:])
            return t
        wrad_sb = load("wrad_rep")
        wmix_sb = load("wmix")
        wsc00_sb = load("wsc00")
        wsc10_sb = load("wsc10")
        wp_sb = load("wp_rep")
        wro0_sb = load("wro0_rep")
        wm1_sb = load("wm1")
        bm1_sb = load("bm1_rep")
        wm2_sb = load("wm2_rep")
        npi_sb = load("npi_rep")
        ixsp_sb = load("idx_spos")
        ixrp_sb = load("idx_rpos")
        ixf1_sb = load("idx_f1")
        ixf2_sb = load("idx_f2")
        ixsa_sb = load("idx_spec_all")
        recv_sb = load("recvloc")
        wemb_sb = load("W_embed")

        ident = const.tile([128, 128], f32, tag="ident")
        make_identity(nc, ident[:])
        identb = const.tile([128, 128], bf16, tag="identb")
        nc.vector.tensor_copy(identb[:], ident[:])
        iota_i = const.tile([128, 128], i32, tag="iota_i")
        nc.gpsimd.iota(iota_i[:], pattern=[[1, 128]], base=0,
                       channel_multiplier=0)
        iota_f = const.tile([128, 128], f32, tag="iota_f")
        nc.vector.tensor_copy(iota_f[:], iota_i[:])
        ones_row = const.tile([1, 128], f32, tag="ones_row")
        nc.gpsimd.memset(ones_row[:], 1.0)
        negpi = const.tile([128, 1], f32, tag="negpi")
        nc.gpsimd.memset(negpi[:], -float(np.pi))

        Y_sb = pers.tile([128, NSUB, 9], bf16, tag="Y")
        feats0_sb = pers.tile([128, NT, 64], f32, tag="feats0")
        f0T_sb = pers.tile([64, NT * 128], f32, tag="f0T")
        oute_sb = pers.tile([128, 2, NT], f32, tag="oute")
        sc1_sb = pers.tile([1, S * 64], f32, tag="sc1")

        IOX = bass.IndirectOffsetOnAxis

        # ---- T0: feats0_init = W_embed[species] for every table slot ----
        for j in range(TBN // 128):
            emb_t = work.tile([128, 64], f32, tag="emb_t")
            nc.gpsimd.indirect_dma_start(
                out=emb_t[:], out_offset=None, in_=din["W_embed"][:],
                in_offset=IOX(ap=ixsa_sb[:, j:j + 1], axis=0))
            nc.sync.dma_start(T0[j * 128:(j + 1) * 128, :], emb_t[:])

        # ---- sc1 table: sc1[s] = W_embed[s] @ W_sc[0,s,0] ----
        ps_wT = psum.tile([64, 16], f32, tag="psS")
        nc.tensor.transpose(ps_wT[:, 0:S], wemb_sb[:], identity=ident[0:S, 0:S])
        wembT = const.tile([64, 16], f32, tag="wembT")
        nc.vector.tensor_copy(wembT[:, 0:S], ps_wT[:, 0:S])
        for s in range(S):
            ps_s1 = psum.tile([1, 64], f32, tag="psS")
            nc.tensor.matmul(ps_s1[:], lhsT=wembT[:, s:s + 1],
                             rhs=wsc00_sb[:, s, :], start=True, stop=True)
            nc.vector.tensor_copy(sc1_sb[0:1, s * 64:(s + 1) * 64], ps_s1[:])

        # ---- geometry (per 8-subtile group of 1024 edges) ----
        for g in range(NGRP):
            ps_g = gwork.tile([128, 8, 64], f32, tag="ps_g")
            pr_g = gwork.tile([128, 8, 64], f32, tag="pr_g")
            for u in range(8):
                gs = g * 8 + u
                nc.gpsimd.indirect_dma_start(
                    out=ps_g[:, u, :], out_offset=None, in_=din["pos_pad"][:],
                    in_offset=IOX(ap=ixsp_sb[:, gs:gs + 1], axis=0))
                nc.gpsimd.indirect_dma_start(
                    out=pr_g[:, u, :], out_offset=None, in_=din["pos_pad"][:],
                    in_offset=IOX(ap=ixrp_sb[:, gs:gs + 1], axis=0))
            vec = gwork.tile([128, 8, 3], f32, tag="vec")
            nc.vector.tensor_tensor(vec[:], pr_g[:, :, 0:3], ps_g[:, :, 0:3],
                                    op=OP.subtract)
            sq = gwork.tile([128, 8, 3], f32, tag="sq")
            nc.scalar.activation(sq[:], vec[:], AF.Square)
            r2 = gwork.tile([128, 8, 1], f32, tag="r2")
            nc.vector.reduce_sum(r2[:], sq[:], axis=AX.X)
            r_ = gwork.tile([128, 8], f32, tag="r_")
            nc.scalar.activation(r_[:], r2[:, :, 0], AF.Sqrt)
            mz = gwork.tile([128, 8], f32, tag="mz")
            nc.vector.tensor_scalar(mz[:], r_[:], 1e-9, None, op0=OP.is_le)
            rs_ = gwork.tile([128, 8], f32, tag="rs_")
            nc.vector.tensor_tensor(rs_[:], r_[:], mz[:], op=OP.add)
            rinv = gwork.tile([128, 8], f32, tag="rinv")
            nc.vector.reciprocal(rinv[:], rs_[:])
            uh = gwork.tile([128, 8, 3], f32, tag="uh")
            nc.vector.tensor_tensor(
                uh[:], vec[:], rinv[:, :, None].to_broadcast([128, 8, 3]),
                op=OP.mult)
            u_ = gwork.tile([128, 8], f32, tag="u_")
            nc.vector.tensor_scalar(u_[:], r_[:], 1.0 / R_MAX, None,
                                    op0=OP.mult)
            ysl = Y_sb[:, g * 8:(g + 1) * 8, :]
            nc.vector.tensor_scalar(ysl[:, :, 0:1], u_[:, :, None], 0.0, 1.0,
                                    op0=OP.mult, op1=OP.add)
            nc.vector.tensor_scalar(ysl[:, :, 1:4], uh[:], S3, None,
                                    op0=OP.mult)
            xs = gwork.tile([128, 8], f32, tag="xs")
            nc.vector.tensor_scalar(xs[:], uh[:, :, 0], S15, None, op0=OP.mult)
            ys = gwork.tile([128, 8], f32, tag="ys")
            nc.vector.tensor_scalar(ys[:], uh[:, :, 1], S15, None, op0=OP.mult)
            nc.vector.tensor_tensor(ysl[:, :, 4], xs[:], uh[:, :, 1],
                                    op=OP.mult)
            nc.vector.tensor_tensor(ysl[:, :, 5], ys[:], uh[:, :, 2],
                                    op=OP.mult)
            nc.vector.tensor_tensor(ysl[:, :, 7], xs[:], uh[:, :, 2],
                                    op=OP.mult)
            z2 = gwork.tile([128, 8], f32, tag="z2")
            nc.vector.tensor_tensor(z2[:], uh[:, :, 2], uh[:, :, 2], op=OP.mult)
            nc.vector.tensor_scalar(ysl[:, :, 6], z2[:], 1.5 * S5, -0.5 * S5,
                                    op0=OP.mult, op1=OP.add)
            x2 = gwork.tile([128, 8], f32, tag="x2")
            nc.vector.tensor_tensor(x2[:], uh[:, :, 0], uh[:, :, 0], op=OP.mult)
            y2 = gwork.tile([128, 8], f32, tag="y2")
            nc.vector.tensor_tensor(y2[:], uh[:, :, 1], uh[:, :, 1], op=OP.mult)
            d2 = gwork.tile([128, 8], f32, tag="d2")
            nc.vector.tensor_tensor(d2[:], x2[:], y2[:], op=OP.subtract)
            nc.vector.tensor_scalar(ysl[:, :, 8], d2[:], 0.5 * S15, None,
                                    op0=OP.mult)
            sarg = gwork.tile([128, 8, NB], f32, tag="sarg")
            nc.vector.tensor_tensor(
                sarg[:], u_[:, :, None].to_broadcast([128, 8, NB]),
                npi_sb[:, None, :].to_broadcast([128, 8, NB]), op=OP.mult)
            t1 = gwork.tile([128, 8, NB], f32, tag="t1")
            nc.vector.tensor_scalar(t1[:], sarg[:], 1.0 / (2.0 * np.pi), None,
                                    op0=OP.mult)
            ki = gwork.tile([128, 8, NB], i32, tag="ki")
            nc.vector.tensor_copy(ki[:], t1[:])
            kf = gwork.tile([128, 8, NB], f32, tag="kf")
            nc.vector.tensor_copy(kf[:], ki[:])
            t2 = gwork.tile([128, 8, NB], f32, tag="t2")
            nc.vector.tensor_scalar(t2[:], kf[:], 2.0 * np.pi, None,
                                    op0=OP.mult)
            y1 = gwork.tile([128, 8, NB], f32, tag="y1")
            nc.vector.tensor_tensor(y1[:], sarg[:], t2[:], op=OP.subtract)
            mgt = gwork.tile([128, 8, NB], f32, tag="mgt")
            nc.vector.tensor_scalar(mgt[:], y1[:], float(np.pi), None,
                                    op0=OP.is_gt)
            t3 = gwork.tile([128, 8, NB], f32, tag="t3")
            nc.vector.tensor_scalar(t3[:], mgt[:], 2.0 * np.pi, None,
                                    op0=OP.mult)
            smod = gwork.tile([128, 8, NB], f32, tag="smod")
            nc.vector.tensor_tensor(smod[:], y1[:], t3[:], op=OP.subtract)
            sn = gwork.tile([128, 8, NB], f32, tag="sn")
            nc.scalar.activation(sn[:], smod[:], AF.Sin)
            bess = gwork.tile([128, 8, NB], f32, tag="bess")
            nc.vector.tensor_tensor(
                bess[:], sn[:], rinv[:, :, None].to_broadcast([128, 8, NB]),
                op=OP.mult)
            u2 = gwork.tile([128, 8], f32, tag="u2")
            nc.vector.tensor_tensor(u2[:], u_[:], u_[:], op=OP.mult)
            u4 = gwork.tile([128, 8], f32, tag="u4")
            nc.vector.tensor_tensor(u4[:], u2[:], u2[:], op=OP.mult)
            u5 = gwork.tile([128, 8], f32, tag="u5")
            nc.vector.tensor_tensor(u5[:], u4[:], u_[:], op=OP.mult)
            tb = gwork.tile([128, 8], f32, tag="tb")
            nc.vector.tensor_scalar(tb[:], u_[:], 35.0, -21.0, op0=OP.mult,
                                    op1=OP.add)
            ta = gwork.tile([128, 8], f32, tag="ta")
            nc.vector.tensor_scalar(ta[:], u2[:], 15.0, None, op0=OP.mult)
            tb2 = gwork.tile([128, 8], f32, tag="tb2")
            nc.vector.tensor_tensor(tb2[:], tb[:], ta[:], op=OP.subtract)
            e0_ = gwork.tile([128, 8], f32, tag="e0_")
            nc.vector.tensor_tensor(e0_[:], u5[:], tb2[:], op=OP.mult)
            e1_ = gwork.tile([128, 8], f32, tag="e1_")
            nc.vector.tensor_scalar(e1_[:], e0_[:], SQ25, SQ25, op0=OP.mult,
                                    op1=OP.add)
            mlt = gwork.tile([128, 8], f32, tag="mlt")
            nc.vector.tensor_scalar(mlt[:], u_[:], 1.0, None, op0=OP.is_lt)
            env = gwork.tile([128, 8], f32, tag="env")
            nc.vector.tensor_tensor(env[:], e1_[:], mlt[:], op=OP.mult)
            rad_g = gwork.tile([128, 8, NB], f32, tag="rad_g")
            nc.vector.tensor_tensor(
                rad_g[:], bess[:], env[:, :, None].to_broadcast([128, 8, NB]),
                op=OP.mult)
            rT_g = gwork.tile([8, 8 * 128], bf16, tag="rT_g")
            for u in range(8):
                ps_rT = psum.tile([8, 128], f32, tag="psS")
                nc.tensor.transpose(ps_rT[:], rad_g[:, u, :],
                                    identity=ident[:])
                nc.vector.tensor_copy(rT_g[:, u * 128:(u + 1) * 128],
                                      ps_rT[:])
            nc.sync.dma_start(radT_d[:, g * 1024:(g + 1) * 1024], rT_g[:])
            if debug and g == 0:
                nc.sync.dma_start(dbg["dbg_radg"][:],
                                  rad_g[:].rearrange("p a b -> p (a b)")[:])
                nc.sync.dma_start(dbg["dbg_sarg"][:],
                                  sarg[:].rearrange("p a b -> p (a b)")[:])
                nc.sync.dma_start(dbg["dbg_sn"][:],
                                  sn[:].rearrange("p a b -> p (a b)")[:])
                nc.sync.dma_start(dbg["dbg_env"][:], env[:])
                nc.sync.dma_start(dbg["dbg_rinv"][:], rinv[:])

        # ---- two interaction layers ----
        for li in range(2):
            ixf_sb = ixf1_sb if li == 0 else ixf2_sb
            Tbl = T0 if li == 0 else T1f
            for t in range(NT):
                hs_g = work.tile([128, ST, 64], f32, tag="hs_g")
                for s_ in range(ST):
                    gs = t * ST + s_
                    nc.gpsimd.indirect_dma_start(
                        out=hs_g[:, s_, :], out_offset=None, in_=Tbl[:],
                        in_offset=IOX(ap=ixf_sb[:, gs:gs + 1], axis=0))
                psA = psA_p.tile([128, 512], f32, tag="psA")
                psA2 = psA_p.tile([128, 64], f32, tag="psA2")
                radT_t = work.tile([8, ST * 128], bf16, tag="radT_t")
                nc.sync.dma_start(
                    radT_t[:], radT_d[:, t * ST * 128:(t + 1) * ST * 128])
                ind_t = work.tile([128, ST, 128], bf16, tag="ind_t")
                nc.vector.tensor_tensor(
                    ind_t[:],
                    iota_f[:, None, :].to_broadcast([128, ST, 128]),
                    recv_sb[:, t * ST:(t + 1) * ST, None].to_broadcast(
                        [128, ST, 128]),
                    op=OP.is_equal)
                for s_ in range(ST):
                    gs = t * ST + s_
                    ps_rl = psum.tile([128, 192], f32, tag="psS")
                    nc.tensor.matmul(
                        ps_rl[:],
                        lhsT=radT_t[:, s_ * 128:(s_ + 1) * 128],
                        rhs=wrad_sb[0:8, li, :],
                        start=True, stop=True)
                    msg = work.tile([128, 576], bf16, tag="msg")
                    z12 = work.tile([128, 2, 64], bf16, tag="z12")
                    nc.vector.tensor_tensor(
                        msg[:, 0:64], hs_g[:, s_, :], ps_rl[:, 0:64],
                        op=OP.mult)
                    nc.vector.tensor_tensor(
                        z12[:],
                        hs_g[:, s_, None, :].to_broadcast([128, 2, 64]),
                        ps_rl[:, 64:192].rearrange("p (l k) -> p l k", l=2)[:],
                        op=OP.mult)
                    nc.vector.tensor_tensor(
                        msg[:, 64:256].rearrange("p (m k) -> p m k", m=3)[:],
                        z12[:, 0, None, :].to_broadcast([128, 3, 64]),
                        Y_sb[:, gs, 1:4][:, :, None].to_broadcast([128, 3, 64]),
                        op=OP.mult)
                    nc.vector.tensor_tensor(
                        msg[:, 256:576].rearrange("p (m k) -> p m k", m=5)[:],
                        z12[:, 1, None, :].to_broadcast([128, 5, 64]),
                        Y_sb[:, gs, 4:9][:, :, None].to_broadcast([128, 5, 64]),
                        op=OP.mult)
                    if debug and li == 0 and t == 0 and s_ == 0:
                        nc.sync.dma_start(dbg["dbg_msg"][:], msg[:])
                    nc.tensor.matmul(psA[:], lhsT=ind_t[:, s_, :], rhs=msg[:, 0:512],
                                     start=(s_ == 0), stop=(s_ == ST - 1))
                    nc.tensor.matmul(psA2[:], lhsT=ind_t[:, s_, :], rhs=msg[:, 512:576],
                                     start=(s_ == 0), stop=(s_ == ST - 1))
                if debug and li == 0 and t == 0:
                    nc.sync.dma_start(dbg["dbg_rad"][:], radT_t[:])
                    nc.sync.dma_start(dbg["dbg_hs"][:],
                                      hs_g[:].rearrange("p a b -> p (a b)")[:])
                    nc.sync.dma_start(dbg["dbg_ind"][:],
                                      ind_t[:].rearrange("p a b -> p (a b)")[:])
                # node phase for tile t
                A_sb = work.tile([128, 576], f32, tag="A_sb")
                nc.vector.tensor_scalar(A_sb[:, 0:512], psA[:], 1.0 / AVG,
                                        None, op0=OP.mult)
                nc.vector.tensor_scalar(A_sb[:, 512:576], psA2[:], 1.0 / AVG,
                                        None, op0=OP.mult)
                if debug and li == 0 and t == 0:
                    nc.sync.dma_start(dbg["dbg_A"][:], A_sb[:])
                Am_sb = work.tile([128, 576], f32, tag="Am_sb")
                for m in range(9):
                    ps_T = psum.tile([64, 128], f32, tag="psS")
                    nc.tensor.transpose(ps_T[:], A_sb[:, m * 64:(m + 1) * 64],
                                        identity=ident[:])
                    AT = work.tile([64, 128], f32, tag="AT")
                    nc.vector.tensor_copy(AT[:], ps_T[:])
                    ps_m = psum.tile([128, 64], f32, tag="psS")
                    nc.tensor.matmul(ps_m[:], lhsT=AT[:],
                                     rhs=wmix_sb[:, li, LM[m], :],
                                     start=True, stop=True)
                    nc.vector.tensor_copy(Am_sb[:, m * 64:(m + 1) * 64],
                                          ps_m[:])
                if debug and li == 0 and t == 0:
                    nc.sync.dma_start(dbg["dbg_Am"][:], Am_sb[:])
                sqt = work.tile([128, 576], f32, tag="sqt")
                nc.scalar.activation(sqt[:], Am_sb[:], AF.Square)
                r1 = work.tile([128, 256], f32, tag="r1")
                nc.vector.tensor_tensor(r1[:], sqt[:, 0:256], sqt[:, 256:512],
                                        op=OP.add)
                r2_ = work.tile([128, 128], f32, tag="r2_")
                nc.vector.tensor_tensor(r2_[:], r1[:, 0:128], r1[:, 128:256],
                                        op=OP.add)
                r3 = work.tile([128, 64], f32, tag="r3")
                nc.vector.tensor_tensor(r3[:], r2_[:, 0:64], r2_[:, 64:128],
                                        op=OP.add)
                inv = work.tile([128, 64], f32, tag="inv")
                nc.vector.tensor_tensor(inv[:], r3[:], sqt[:, 512:576],
                                        op=OP.add)
                fa = work.tile([128, 64], f32, tag="fa")
                nc.vector.tensor_tensor(fa[:], wp_sb[:, li, 1, :],
                                        Am_sb[:, 0:64], op=OP.mult)
                fb = work.tile([128, 64], f32, tag="fb")
                nc.vector.tensor_tensor(fb[:], wp_sb[:, li, 2, :], inv[:],
                                        op=OP.mult)
                fc_ = work.tile([128, 64], f32, tag="fc_")
                nc.vector.tensor_tensor(fc_[:], fa[:], fb[:], op=OP.add)
                fw = work.tile([128, 64], f32, tag="fw")
                nc.vector.tensor_tensor(fw[:], fc_[:], wp_sb[:, li, 0, :],
                                        op=OP.add)
                B0 = work.tile([128, 64], f32, tag="B0")
                nc.vector.tensor_tensor(B0[:], Am_sb[:, 0:64], fw[:],
                                        op=OP.mult)
                ps_sc = psum.tile([128, 64], f32, tag="psS")
                if li == 0:
                    nc.tensor.matmul(
                        ps_sc[:], lhsT=ones_row[:],
                        rhs=sc1_sb[0:1, (t // 2) * 64:(t // 2 + 1) * 64],
                        start=True, stop=True)
                else:
                    nc.tensor.matmul(ps_sc[:],
                                     lhsT=f0T_sb[:, t * 128:(t + 1) * 128],
                                     rhs=wsc10_sb[:, t // 2, :],
                                     start=True, stop=True)
                fnew = feats0_sb[:, t, :]
                nc.vector.tensor_tensor(fnew[:], B0[:], ps_sc[:], op=OP.add)
                if li == 0:
                    mro = work.tile([128, 64], f32, tag="mro")
                    nc.vector.tensor_tensor(mro[:], fnew[:], wro0_sb[:],
                                            op=OP.mult)
                    nc.vector.reduce_sum(oute_sb[:, 0, t:t + 1], mro[:],
                                         axis=AX.X)
                else:
                    ps_fT = psum.tile([64, 128], f32, tag="psS")
                    nc.tensor.transpose(ps_fT[:], fnew[:], identity=ident[:])
                    fT = work.tile([64, 128], f32, tag="AT")
                    nc.vector.tensor_copy(fT[:], ps_fT[:])
                    ps_h = psum.tile([128, 16], f32, tag="psS")
                    nc.tensor.matmul(ps_h[:], lhsT=fT[:], rhs=wm1_sb[:],
                                     start=True, stop=True)
                    hb = work.tile([128, 16], f32, tag="hb")
                    nc.vector.tensor_tensor(hb[:], ps_h[:], bm1_sb[:],
                                            op=OP.add)
                    hsg = work.tile([128, 16], f32, tag="hsg")
                    nc.scalar.activation(hsg[:], hb[:], AF.Silu)
                    m2 = work.tile([128, 16], f32, tag="m2")
                    nc.vector.tensor_tensor(m2[:], hsg[:], wm2_sb[:],
                                            op=OP.mult)
                    nc.vector.reduce_sum(oute_sb[:, 1, t:t + 1], m2[:],
                                         axis=AX.X)
            if li == 0:
                if debug:
                    nc.sync.dma_start(
                        dbg["dbg_Y"][:],
                        Y_sb[:].rearrange("p a b -> p (a b)")[:])
                    nc.sync.dma_start(
                        dbg["dbg_f0"][:],
                        feats0_sb[:].rearrange("p t k -> p (t k)")[:])
                nc.sync.dma_start(
                    T1s[:].rearrange("(p t) k -> p (t k)", p=128)[:],
                    feats0_sb[:].rearrange("p t k -> p (t k)")[:])
                nc.gpsimd.collective_compute(
                    "AllGather", mybir.AluOpType.bypass,
                    ins=[T1s[:].opt()], outs=[T1f[:].opt()],
                    replica_groups=[list(range(NCORE))])
                for t in range(NT):
                    ps_fT = psum.tile([64, 128], f32, tag="psS")
                    nc.tensor.transpose(ps_fT[:], feats0_sb[:, t, :],
                                        identity=ident[:])
                    nc.vector.tensor_copy(f0T_sb[:, t * 128:(t + 1) * 128],
                                          ps_fT[:])
        nc.sync.dma_start(out_e[:].rearrange("p a t -> p (a t)")[:],
                          oute_sb[:].rearrange("p a t -> p (a t)")[:])

    nc.compile()
    _prog_cache[("nc", debug)] = nc
    return nc


def _host_prep(inputs):
    pos = np.asarray(inputs["positions"], np.float32)
    spec = np.asarray(inputs["species"]).astype(np.int64)
    snd = np.asarray(inputs["senders"]).astype(np.int64)
    rcv = np.asarray(inputs["receivers"]).astype(np.int64)
    W_embed = np.asarray(inputs["W_embed"], np.float32)
    W_rad = np.asarray(inputs["W_rad"], np.float32)
    W_mix = np.asarray(inputs["W_mix"], np.float32)
    W_prod = np.asarray(inputs["W_prod"], np.float32)
    W_sc = np.asarray(inputs["W_sc"], np.float32)
    W_ro0 = np.asarray(inputs["W_ro0"], np.float32)
    W_m1 = np.asarray(inputs["W_m1"], np.float32)
    b_m1 = np.asarray(inputs["b_m1"], np.float32)
    W_m2 = np.asarray(inputs["W_m2"], np.float32)

    core_of = np.arange(N) // NPC
    slot_of = np.empty(N, np.int64)
    slot2node = -np.ones((NCORE, NSLOT), np.int64)
    for c in range(NCORE):
        nodes = np.arange(c * NPC, (c + 1) * NPC)
        sp = spec[nodes]
        for s in range(S):
            ns = nodes[sp == s]
            assert len(ns) <= SPS, f"species overflow core {c} s {s}: {len(ns)}"
            slot_of[ns] = s * SPS + np.arange(len(ns))
            slot2node[c, s * SPS:s * SPS + len(ns)] = ns

    gslot = core_of * NSLOT + slot_of
    t1row = core_of * NSLOT + (slot_of % 128) * NT + slot_of // 128
    ecore = core_of[rcv]
    eslot = slot_of[rcv]
    etile = eslot // 128

    idx_spos = np.zeros((NCORE, 128, NSUB), np.int32)
    idx_rpos = np.zeros((NCORE, 128, NSUB), np.int32)
    idx_f1 = np.zeros((NCORE, 128, NSUB), np.int32)
    idx_f2 = np.zeros((NCORE, 128, NSUB), np.int32)
    recvloc = np.zeros((NCORE, 128, NSUB), np.float32)

    for c in range(NCORE):
        in_c = np.nonzero(ecore == c)[0]
        t_c = etile[in_c]
        for t in range(NT):
            ee = in_c[t_c == t]
            cnt = len(ee)
            assert cnt <= ST * 128, f"tile overflow c{c} t{t}: {cnt}"
            sl = t * ST * 128 + np.arange(cnt)
            p, col = sl % 128, sl // 128
            idx_spos[c, p, col] = snd[ee]
            idx_rpos[c, p, col] = rcv[ee]
            idx_f1[c, p, col] = gslot[snd[ee]]
            idx_f2[c, p, col] = t1row[snd[ee]]
            recvloc[c, p, col] = (eslot[ee] % 128).astype(np.float32)

    pos_pad = np.zeros((N, 64), np.float32)
    pos_pad[:, 0:3] = pos

    wrad_rep = np.zeros((128, 2, 192), np.float32)
    for i in range(2):
        flat = W_rad[i].transpose(1, 0, 2).reshape(NB, 192)   # [b, (l k)]
        wrad_rep[:, i, :] = np.tile(flat, (16, 1))
    wrad_rep = wrad_rep.astype(ml_dtypes.bfloat16)

    wp_rep = np.zeros((128, 2, 3, 64), np.float32)
    for i in range(2):
        for j in range(3):
            wp_rep[:, i, j, :] = W_prod[i, j, 0][None, :]

    npi_rep = np.tile(
        (np.arange(1, NB + 1, dtype=np.float32) * np.float32(np.pi))[None, :],
        (128, 1))

    spec_slot = np.empty((NCORE, NSLOT), np.int32)
    for c in range(NCORE):
        for s in range(S):
            blk = slot2node[c, s * SPS:(s + 1) * SPS]
            v = np.full(SPS, s, np.int32)
            v[blk >= 0] = spec[blk[blk >= 0]]
            spec_slot[c, s * SPS:(s + 1) * SPS] = v
    gspec = spec_slot.reshape(-1)
    idx_spec_all = np.ascontiguousarray(
        gspec.reshape(TBN // 128, 128).T.astype(np.int32))

    shared = dict(
        pos_pad=pos_pad,
        W_embed=W_embed,
        wrad_rep=wrad_rep,
        wmix=np.ascontiguousarray(W_mix.transpose(2, 0, 1, 3)),
        wsc00=np.ascontiguousarray(W_sc[0, :, 0].transpose(1, 0, 2)),
        wsc10=np.ascontiguousarray(W_sc[1, :, 0].transpose(1, 0, 2)),
        wp_rep=wp_rep,
        wro0_rep=np.tile(W_ro0[None, :], (128, 1)).astype(np.float32),
        wm1=W_m1,
        bm1_rep=np.tile(b_m1[None, :], (128, 1)).astype(np.float32),
        wm2_rep=np.tile(W_m2[None, :], (128, 1)).astype(np.float32),
        npi_rep=npi_rep,
        idx_spec_all=idx_spec_all,
    )
    in_maps = []
    for c in range(NCORE):
        m = dict(shared)
        m["idx_spos"] = idx_spos[c]
        m["idx_rpos"] = idx_rpos[c]
        m["idx_f1"] = idx_f1[c]
        m["idx_f2"] = idx_f2[c]
        m["recvloc"] = recvloc[c]
        in_maps.append(m)
    return in_maps, slot2node


def kernel(**inputs):
    from concourse import bass_utils
    in_maps, slot2node = _host_prep(inputs)
    nc = _build_program()
    res = bass_utils.run_bass_kernel_spmd(nc, in_maps,
                                          core_ids=list(range(NCORE)))
    out = np.zeros((N, 2), np.float32)
    for c in range(NCORE):
        oe = np.asarray(res.results[c]["out_e"])      # [128, 2, NT]
        for i in range(2):
            vals = oe[:, i, :].T.reshape(-1)          # slot-major
            valid = slot2node[c] >= 0
            out[slot2node[c][valid], i] = vals[valid]
    return out



# revision 9
# speedup vs baseline: 2.1471x; 2.1471x over previous
"""EnergyMACE TRN2 kernel v2: edge/graph-parallel over 8 NeuronCores.

vs baseline: balanced 16-tile node binning (ST~17 vs 18x20 subtiles),
layer-0 sender features via one-hot species matmul (no gather, no T0),
host-packed edge endpoint positions (no position gathers), bf16 matmuls,
single Sqrt table phase + one sin/silu/square table, fused radial chain,
species self-connections via host-fused tables and select-sum.
All transposes/matmuls use baseline-proven base-0 patterns.
"""
import sys
import numpy as np

for p in ("/opt/trn_rl_repo", "/root/.axon_site/_ro/trn_rl_repo"):
    if p not in sys.path:
        sys.path.insert(0, p)

import ml_dtypes  # noqa: E402

N, E, S, K, NB = 16384, 262144, 10, 64, 8
R_MAX, AVG = 5.0, 16.0
NCORE = 8
NT = 16
NPC = N // NCORE
MLP_H = 16

S3 = float(np.sqrt(3.0, dtype=np.float32))
S15 = float(np.sqrt(15.0, dtype=np.float32))
S5 = float(np.sqrt(5.0, dtype=np.float32))
SQ25 = float(np.float32(np.sqrt(2.0 / R_MAX)))
PI = float(np.pi)
LM = [0, 1, 1, 1, 2, 2, 2, 2, 2]

_prog_cache = {}


def _build_program(st):
    key = ("nc", st)
    if key in _prog_cache:
        return _prog_cache[key]
    from contextlib import ExitStack
    from concourse import bass, bacc, mybir, tile
    from concourse.masks import make_identity

    ST = st
    NSUB = NT * ST

    f32 = mybir.dt.float32
    bf16 = mybir.dt.bfloat16
    i32 = mybir.dt.int32
    AF = mybir.ActivationFunctionType
    OP = mybir.AluOpType
    AX = mybir.AxisListType

    nc = bacc.Bacc("TRN2", target_bir_lowering=False, debug=False,
                   num_devices=NCORE)

    din = {}

    def inp(name, shape, dt):
        din[name] = nc.dram_tensor(name, shape, dt, kind="ExternalInput").ap()

    inp("es_d", [128, NSUB * 3], f32)
    inp("er_d", [128, NSUB * 3], f32)
    inp("soh_d", [10, NSUB * 128], bf16)
    inp("recv_d", [128, NSUB], bf16)
    inp("idxf2_d", [128, NSUB], i32)
    inp("ohT", [10, NT * 128], bf16)
    inp("ohcols", [128, NT * 10], f32)
    inp("wrad_r", [8, 2, 192], bf16)
    inp("wmix_r", [64, 2, 3, 64], bf16)
    inp("wemb_b", [10, 64], bf16)
    inp("sc0tab", [10, 64], bf16)
    inp("wall", [64, 640], bf16)
    inp("wp_rep", [128, 2, 3, 64], f32)
    inp("wro0_rep", [128, 64], f32)
    inp("wm1_b", [64, MLP_H], bf16)
    inp("bm1_rep", [128, MLP_H], f32)
    inp("wm2_rep", [128, MLP_H], f32)
    inp("npi_rep", [128, NB], f32)
    inp("nh_rep", [128, NB], f32)

    out_e = nc.dram_tensor("out_e", [128, 2, NT], f32,
                           kind="ExternalOutput").ap()

    T1s = nc.dram_tensor("T1s", [NPC, 64], f32, kind="Internal").ap()
    T1f = nc.dram_tensor("T1f", [N, 64], f32, kind="Internal",
                         addr_space="Shared").ap()

    IOX = bass.IndirectOffsetOnAxis

    with tile.TileContext(nc) as tc, ExitStack() as ctx:
        const = ctx.enter_context(tc.tile_pool(name="const", bufs=1))
        pers = ctx.enter_context(tc.tile_pool(name="pers", bufs=1))
        gwork = ctx.enter_context(tc.tile_pool(name="gwork", bufs=2))
        work = ctx.enter_context(tc.tile_pool(name="work", bufs=2))
        nwork = ctx.enter_context(tc.tile_pool(name="nwork", bufs=2))
        psA_p = ctx.enter_context(tc.tile_pool(name="psA", bufs=2,
                                               space="PSUM"))
        psAm_p = ctx.enter_context(tc.tile_pool(name="psAm", bufs=1,
                                                space="PSUM"))
        psS_p = ctx.enter_context(tc.tile_pool(name="psS", bufs=2,
                                               space="PSUM"))

        def load(name):
            src = din[name]
            t = const.tile(list(src.shape), src.dtype, tag=name)
            nc.sync.dma_start(t[:], src[:])
            return t

        recv_sb = load("recv_d")
        idxf2_sb = load("idxf2_d")
        ohT_sb = load("ohT")
        ohcols_sb = load("ohcols")
        wrad_sb = load("wrad_r")
        wmix_sb = load("wmix_r")
        wemb_sb = load("wemb_b")
        sc0tab_sb = load("sc0tab")
        wall_sb = load("wall")
        wp_sb = load("wp_rep")
        wro0_sb = load("wro0_rep")
        wm1_sb = load("wm1_b")
        bm1_sb = load("bm1_rep")
        wm2_sb = load("wm2_rep")
        npi_sb = load("npi_rep")
        nh_sb = load("nh_rep")

        ident = const.tile([128, 128], f32, tag="ident")
        make_identity(nc, ident[:])
        iota_i = const.tile([128, 128], i32, tag="iota_i")
        nc.gpsimd.iota(iota_i[:], pattern=[[1, 128]], base=0,
                       channel_multiplier=0)
        iotab = const.tile([128, 128], bf16, tag="iotab")
        nc.vector.tensor_copy(iotab[:], iota_i[:])

        u_all = pers.tile([128, NT, ST], f32, tag="u_all")
        rinv_all = pers.tile([128, NT, ST], f32, tag="rinv_all")
        uh_all = pers.tile([128, NT, ST, 3], f32, tag="uh_all")
        Y_sb = pers.tile([128, NSUB, 8], bf16, tag="Y")
        radT_all = pers.tile([8, NT, ST * 128], bf16, tag="radT")
        feats0 = pers.tile([128, NT, 64], f32, tag="feats0")
        sc1_sb = pers.tile([128, NT, 64], f32, tag="sc1")
        t1stage = pers.tile([128, NT, 64], f32, tag="t1stage")
        fT_all = pers.tile([64, NT * 128], bf16, tag="fT_all")
        oute_sb = pers.tile([128, 2, NT], f32, tag="oute")

        # ---- geometry pass A (Sqrt phase) ----
        for t in range(NT):
            es_t = gwork.tile([128, ST, 3], f32, tag="es_t")
            nc.sync.dma_start(
                es_t[:].rearrange("p a b -> p (a b)")[:],
                din["es_d"][:, t * ST * 3:(t + 1) * ST * 3])
            er_t = gwork.tile([128, ST, 3], f32, tag="er_t")
            nc.sync.dma_start(
                er_t[:].rearrange("p a b -> p (a b)")[:],
                din["er_d"][:, t * ST * 3:(t + 1) * ST * 3])
            vec = gwork.tile([128, ST, 3], f32, tag="vec")
            nc.vector.tensor_tensor(vec[:], er_t[:], es_t[:], op=OP.subtract)
            sq = gwork.tile([128, ST, 3], f32, tag="sq")
            nc.vector.tensor_tensor(sq[:], vec[:], vec[:], op=OP.mult)
            r2 = gwork.tile([128, ST, 1], f32, tag="r2")
            nc.vector.reduce_sum(r2[:], sq[:], axis=AX.X)
            r_ = gwork.tile([128, ST], f32, tag="r_")
            nc.scalar.activation(r_[:], r2[:, :, 0], AF.Sqrt)
            mz = gwork.tile([128, ST], f32, tag="mz")
            nc.vector.tensor_scalar(mz[:], r_[:], 1e-9, None, op0=OP.is_le)
            rs_ = gwork.tile([128, ST], f32, tag="rs_")
            nc.vector.tensor_tensor(rs_[:], r_[:], mz[:], op=OP.add)
            nc.vector.reciprocal(rinv_all[:, t, :], rs_[:])
            nc.vector.tensor_scalar(u_all[:, t, :], r_[:], 1.0 / R_MAX, None,
                                    op0=OP.mult)
            nc.vector.tensor_tensor(
                uh_all[:, t, :, :], vec[:],
                rinv_all[:, t, :, None].to_broadcast([128, ST, 3]), op=OP.mult)

        # ---- per tile: geometry pass B + layer 0 ----
        for t in range(NT):
            u_t = u_all[:, t, :]
            rinv_t = rinv_all[:, t, :]
            uh = uh_all[:, t, :, :]
            ysl = Y_sb[:, t * ST:(t + 1) * ST, :]
            nc.vector.tensor_scalar(ysl[:, :, 0:3], uh[:], S3, None,
                                    op0=OP.mult)
            xs = gwork.tile([128, ST], f32, tag="xs")
            nc.vector.tensor_scalar(xs[:], uh[:, :, 0], S15, None, op0=OP.mult)
            ys = gwork.tile([128, ST], f32, tag="ys")
            nc.vector.tensor_scalar(ys[:], uh[:, :, 1], S15, None, op0=OP.mult)
            nc.vector.tensor_tensor(ysl[:, :, 3], xs[:], uh[:, :, 1],
                                    op=OP.mult)
            nc.vector.tensor_tensor(ysl[:, :, 4], ys[:], uh[:, :, 2],
                                    op=OP.mult)
            z2 = gwork.tile([128, ST], f32, tag="z2")
            nc.vector.tensor_tensor(z2[:], uh[:, :, 2], uh[:, :, 2],
                                    op=OP.mult)
            nc.vector.tensor_scalar(ysl[:, :, 5], z2[:], 1.5 * S5, -0.5 * S5,
                                    op0=OP.mult, op1=OP.add)
            nc.vector.tensor_tensor(ysl[:, :, 6], xs[:], uh[:, :, 2],
                                    op=OP.mult)
            x2 = gwork.tile([128, ST], f32, tag="x2")
            nc.vector.tensor_tensor(x2[:], uh[:, :, 0], uh[:, :, 0],
                                    op=OP.mult)
            y2 = gwork.tile([128, ST], f32, tag="y2")
            nc.vector.tensor_tensor(y2[:], uh[:, :, 1], uh[:, :, 1],
                                    op=OP.mult)
            d2 = gwork.tile([128, ST], f32, tag="d2")
            nc.vector.tensor_tensor(d2[:], x2[:], y2[:], op=OP.subtract)
            nc.vector.tensor_scalar(ysl[:, :, 7], d2[:], 0.5 * S15, None,
                                    op0=OP.mult)
            sarg = gwork.tile([128, ST, NB], f32, tag="sarg")
            nc.vector.tensor_tensor(
                sarg[:], u_t[:, :, None].to_broadcast([128, ST, NB]),
                npi_sb[:, None, :].to_broadcast([128, ST, NB]), op=OP.mult)
            t1 = gwork.tile([128, ST, NB], f32, tag="t1")
            nc.vector.tensor_tensor(
                t1[:], u_t[:, :, None].to_broadcast([128, ST, NB]),
                nh_sb[:, None, :].to_broadcast([128, ST, NB]), op=OP.mult)
            ki = gwork.tile([128, ST, NB], i32, tag="ki")
            nc.vector.tensor_copy(ki[:], t1[:])
            kf = gwork.tile([128, ST, NB], f32, tag="kf")
            nc.vector.tensor_copy(kf[:], ki[:])
            y1 = gwork.tile([128, ST, NB], f32, tag="y1")
            nc.vector.tensor_scalar(y1[:], kf[:], -2.0 * PI, None,
                                    op0=OP.mult)
            nc.vector.tensor_tensor(y1[:], y1[:], sarg[:], op=OP.add)
            mgt = gwork.tile([128, ST, NB], f32, tag="mgt")
            nc.vector.tensor_scalar(mgt[:], y1[:], PI, None, op0=OP.is_gt)
            t3 = gwork.tile([128, ST, NB], f32, tag="t3")
            nc.vector.tensor_scalar(t3[:], mgt[:], 2.0 * PI, None,
                                    op0=OP.mult)
            yw = gwork.tile([128, ST, NB], f32, tag="yw")
            nc.vector.tensor_tensor(yw[:], y1[:], t3[:], op=OP.subtract)
            sn = gwork.tile([128, ST, NB], f32, tag="sn")
            nc.scalar.activation(sn[:], yw[:], AF.Sin)
            u2 = gwork.tile([128, ST], f32, tag="u2")
            nc.vector.tensor_tensor(u2[:], u_t[:], u_t[:], op=OP.mult)
            u4 = gwork.tile([128, ST], f32, tag="u4")
            nc.vector.tensor_tensor(u4[:], u2[:], u2[:], op=OP.mult)
            u5 = gwork.tile([128, ST], f32, tag="u5")
            nc.vector.tensor_tensor(u5[:], u4[:], u_t[:], op=OP.mult)
            tb = gwork.tile([128, ST], f32, tag="tb")
            nc.vector.tensor_scalar(tb[:], u_t[:], 35.0, -21.0, op0=OP.mult,
                                    op1=OP.add)
            ta = gwork.tile([128, ST], f32, tag="ta")
            nc.vector.tensor_scalar(ta[:], u2[:], 15.0, None, op0=OP.mult)
            tb2 = gwork.tile([128, ST], f32, tag="tb2")
            nc.vector.tensor_tensor(tb2[:], tb[:], ta[:], op=OP.subtract)
            e0_ = gwork.tile([128, ST], f32, tag="e0_")
            nc.vector.tensor_tensor(e0_[:], u5[:], tb2[:], op=OP.mult)
            e1_ = gwork.tile([128, ST], f32, tag="e1_")
            nc.vector.tensor_scalar(e1_[:], e0_[:], SQ25, SQ25, op0=OP.mult,
                                    op1=OP.add)
            mlt = gwork.tile([128, ST], f32, tag="mlt")
            nc.vector.tensor_scalar(mlt[:], u_t[:], 1.0, None, op0=OP.is_lt)
            env = gwork.tile([128, ST], f32, tag="env")
            nc.vector.tensor_tensor(env[:], e1_[:], mlt[:], op=OP.mult)
            renv = gwork.tile([128, ST], f32, tag="renv")
            nc.vector.tensor_tensor(renv[:], env[:], rinv_t[:], op=OP.mult)
            rad3 = gwork.tile([128, ST, NB], f32, tag="rad3")
            nc.vector.tensor_tensor(
                rad3[:], sn[:], renv[:, :, None].to_broadcast([128, ST, NB]),
                op=OP.mult)
            for s_ in range(ST):
                ps_t = psS_p.tile([128, 256], f32, tag="ps_c")
                nc.tensor.transpose(ps_t[0:8, 0:128], rad3[:, s_, :],
                                    identity=ident[:])
                nc.vector.tensor_copy(
                    radT_all[:, t, s_ * 128:(s_ + 1) * 128],
                    ps_t[0:8, 0:128])

            # ---- layer 0 ----
            soh_t = work.tile([10, ST * 128], bf16, tag="soh_t")
            nc.sync.dma_start(
                soh_t[:], din["soh_d"][:, t * ST * 128:(t + 1) * ST * 128])
            ind_t = work.tile([128, ST, 128], bf16, tag="ind_t")
            nc.vector.tensor_tensor(
                ind_t[:], iotab[:, None, :].to_broadcast([128, ST, 128]),
                recv_sb[:, t * ST:(t + 1) * ST, None].to_broadcast(
                    [128, ST, 128]),
                op=OP.is_equal)
            psA = psA_p.tile([128, 512], f32, tag="psA")
            psA2 = psA_p.tile([128, 64], f32, tag="psA2")
            for s_ in range(ST):
                gs = t * ST + s_
                ps_c = psS_p.tile([128, 256], f32, tag="ps_c")
                nc.tensor.matmul(ps_c[:, 0:64],
                                 lhsT=soh_t[:, s_ * 128:(s_ + 1) * 128],
                                 rhs=wemb_sb[:], start=True, stop=True)
                nc.tensor.matmul(
                    ps_c[:, 64:256],
                    lhsT=radT_all[:, t, s_ * 128:(s_ + 1) * 128],
                    rhs=wrad_sb[:, 0, :], start=True, stop=True)
                hs0_sb = work.tile([128, 64], bf16, tag="hs0_sb")
                nc.vector.tensor_copy(hs0_sb[:], ps_c[:, 0:64])
                msg = work.tile([128, 576], bf16, tag="msg")
                z12 = work.tile([128, 2, 64], bf16, tag="z12")
                nc.vector.tensor_tensor(msg[:, 0:64], hs0_sb[:],
                                        ps_c[:, 64:128], op=OP.mult)
                nc.vector.tensor_tensor(
                    z12[:], hs0_sb[:, None, :].to_broadcast([128, 2, 64]),
                    ps_c[:, 128:256].rearrange("p (l k) -> p l k", l=2)[:],
                    op=OP.mult)
                nc.vector.tensor_tensor(
                    msg[:, 64:256].rearrange("p (m k) -> p m k", m=3)[:],
                    z12[:, 0, None, :].to_broadcast([128, 3, 64]),
                    Y_sb[:, gs, 0:3][:, :, None].to_broadcast([128, 3, 64]),
                    op=OP.mult)
                nc.vector.tensor_tensor(
                    msg[:, 256:576].rearrange("p (m k) -> p m k", m=5)[:],
                    z12[:, 1, None, :].to_broadcast([128, 5, 64]),
                    Y_sb[:, gs, 3:8][:, :, None].to_broadcast([128, 5, 64]),
                    op=OP.mult)
                nc.tensor.matmul(psA[:], lhsT=ind_t[:, s_, :],
                                 rhs=msg[:, 0:512],
                                 start=(s_ == 0), stop=(s_ == ST - 1))
                nc.tensor.matmul(psA2[:], lhsT=ind_t[:, s_, :],
                                 rhs=msg[:, 512:576],
                                 start=(s_ == 0), stop=(s_ == ST - 1))
            _node_phase(nc, 0, t, psA, psA2, nwork, psAm_p, psS_p, ident,
                        wmix_sb, wp_sb, ohT_sb, sc0tab_sb, ohcols_sb,
                        wro0_sb, wm1_sb, bm1_sb, wm2_sb,
                        feats0, fT_all, sc1_sb, t1stage, oute_sb,
                        f32, bf16, OP, AF, AX)

        # ---- exchange feats ----
        nc.sync.dma_start(
            T1s[:].rearrange("(p t) k -> p (t k)", p=128)[:],
            t1stage[:].rearrange("p t k -> p (t k)")[:])
        nc.gpsimd.collective_compute(
            "AllGather", mybir.AluOpType.bypass,
            ins=[T1s[:].opt()], outs=[T1f[:].opt()],
            replica_groups=[list(range(NCORE))])

        # sc1 prep: overlaps the AllGather; reuses the mix psum banks
        for t in range(NT):
            psP = psAm_p.tile([128, 512], f32, tag="psAm")
            psP2 = psAm_p.tile([128, 128], f32, tag="psAm2")
            nc.tensor.matmul(psP[:], lhsT=fT_all[:, t * 128:(t + 1) * 128],
                             rhs=wall_sb[:, 0:512], start=True, stop=True)
            nc.tensor.matmul(psP2[:], lhsT=fT_all[:, t * 128:(t + 1) * 128],
                             rhs=wall_sb[:, 512:640], start=True, stop=True)
            acc = sc1_sb[:, t, :]
            nc.vector.tensor_tensor(
                acc[:], psP[:, 0:64],
                ohcols_sb[:, t * 10:t * 10 + 1].to_broadcast([128, 64]),
                op=OP.mult)
            for s in range(1, 10):
                src_ap = psP[:, s * 64:(s + 1) * 64] if s < 8 else \
                    psP2[:, (s - 8) * 64:(s - 7) * 64]
                nc.vector.scalar_tensor_tensor(
                    acc[:], src_ap, ohcols_sb[:, t * 10 + s:t * 10 + s + 1],
                    acc[:], op0=OP.mult, op1=OP.add)

        # ---- layer 1 ----
        for t in range(NT):
            hs_g = work.tile([128, ST, 64], f32, tag="hs_g")
            for s_ in range(ST):
                gs = t * ST + s_
                nc.gpsimd.indirect_dma_start(
                    out=hs_g[:, s_, :], out_offset=None, in_=T1f[:],
                    in_offset=IOX(ap=idxf2_sb[:, gs:gs + 1], axis=0))
            ind_t = work.tile([128, ST, 128], bf16, tag="ind_t")
            nc.vector.tensor_tensor(
                ind_t[:], iotab[:, None, :].to_broadcast([128, ST, 128]),
                recv_sb[:, t * ST:(t + 1) * ST, None].to_broadcast(
                    [128, ST, 128]),
                op=OP.is_equal)
            psA = psA_p.tile([128, 512], f32, tag="psA")
            psA2 = psA_p.tile([128, 64], f32, tag="psA2")
            for s_ in range(ST):
                gs = t * ST + s_
                ps_c = psS_p.tile([128, 256], f32, tag="ps_c")
                nc.tensor.matmul(
                    ps_c[:, 64:256],
                    lhsT=radT_all[:, t, s_ * 128:(s_ + 1) * 128],
                    rhs=wrad_sb[:, 1, :], start=True, stop=True)
                msg = work.tile([128, 576], bf16, tag="msg")
                z12 = work.tile([128, 2, 64], bf16, tag="z12")
                nc.vector.tensor_tensor(msg[:, 0:64], hs_g[:, s_, :],
                                        ps_c[:, 64:128], op=OP.mult)
                nc.vector.tensor_tensor(
                    z12[:], hs_g[:, s_, None, :].to_broadcast([128, 2, 64]),
                    ps_c[:, 128:256].rearrange("p (l k) -> p l k", l=2)[:],
                    op=OP.mult)
                nc.vector.tensor_tensor(
                    msg[:, 64:256].rearrange("p (m k) -> p m k", m=3)[:],
                    z12[:, 0, None, :].to_broadcast([128, 3, 64]),
                    Y_sb[:, gs, 0:3][:, :, None].to_broadcast([128, 3, 64]),
                    op=OP.mult)
                nc.vector.tensor_tensor(
                    msg[:, 256:576].rearrange("p (m k) -> p m k", m=5)[:],
                    z12[:, 1, None, :].to_broadcast([128, 5, 64]),
                    Y_sb[:, gs, 3:8][:, :, None].to_broadcast([128, 5, 64]),
                    op=OP.mult)
                nc.tensor.matmul(psA[:], lhsT=ind_t[:, s_, :],
                                 rhs=msg[:, 0:512],
                                 start=(s_ == 0), stop=(s_ == ST - 1))
                nc.tensor.matmul(psA2[:], lhsT=ind_t[:, s_, :],
                                 rhs=msg[:, 512:576],
                                 start=(s_ == 0), stop=(s_ == ST - 1))
            _node_phase(nc, 1, t, psA, psA2, nwork, psAm_p, psS_p, ident,
                        wmix_sb, wp_sb, ohT_sb, sc0tab_sb, ohcols_sb,
                        wro0_sb, wm1_sb, bm1_sb, wm2_sb,
                        feats0, fT_all, sc1_sb, t1stage, oute_sb,
                        f32, bf16, OP, AF, AX)

        nc.sync.dma_start(out_e[:].rearrange("p a t -> p (a t)")[:],
                          oute_sb[:].rearrange("p a t -> p (a t)")[:])

    nc.compile()
    _prog_cache[key] = nc
    return nc


def _node_phase(nc, li, t, psA, psA2, nwork, psAm_p, psS_p, ident,
                wmix_sb, wp_sb, ohT_sb, sc0tab_sb, ohcols_sb,
                wro0_sb, wm1_sb, bm1_sb, wm2_sb,
                feats0, fT_all, sc1_sb, t1stage, oute_sb,
                f32, bf16, OP, AF, AX):
    A_sb = nwork.tile([128, 576], f32, tag="A_sb")
    nc.vector.tensor_copy(A_sb[:, 0:512], psA[:])
    nc.vector.tensor_copy(A_sb[:, 512:576], psA2[:])
    psAm = psAm_p.tile([128, 512], f32, tag="psAm")
    psAm2 = psAm_p.tile([128, 128], f32, tag="psAm2")
    for m in range(9):
        ps_t = psS_p.tile([128, 256], f32, tag="ps_c")
        nc.tensor.transpose(ps_t[0:64, 0:128], A_sb[:, m * 64:(m + 1) * 64],
                            identity=ident[:])
        AT = nwork.tile([64, 128], bf16, tag="AT")
        nc.vector.tensor_copy(AT[:], ps_t[0:64, 0:128])
        out_ap = psAm[:, (m * 64):(m + 1) * 64] if m < 8 else psAm2[:, 0:64]
        nc.tensor.matmul(out_ap, lhsT=AT[:], rhs=wmix_sb[:, li, LM[m], :],
                         start=True, stop=True)
    Am_sb = nwork.tile([128, 576], f32, tag="Am_sb")
    nc.vector.tensor_copy(Am_sb[:, 0:512], psAm[:])
    nc.vector.tensor_copy(Am_sb[:, 512:576], psAm2[:, 0:64])
    sqt = nwork.tile([128, 576], f32, tag="sqt")
    nc.scalar.activation(sqt[:], Am_sb[:], AF.Square)
    r1 = nwork.tile([128, 256], f32, tag="r1")
    nc.vector.tensor_tensor(r1[:], sqt[:, 0:256], sqt[:, 256:512], op=OP.add)
    r2_ = nwork.tile([128, 128], f32, tag="r2_")
    nc.vector.tensor_tensor(r2_[:], r1[:, 0:128], r1[:, 128:256], op=OP.add)
    r3 = nwork.tile([128, 64], f32, tag="r3")
    nc.vector.tensor_tensor(r3[:], r2_[:, 0:64], r2_[:, 64:128], op=OP.add)
    inv = nwork.tile([128, 64], f32, tag="inv")
    nc.vector.tensor_tensor(inv[:], r3[:], sqt[:, 512:576], op=OP.add)
    fa = nwork.tile([128, 64], f32, tag="fa")
    nc.vector.tensor_tensor(fa[:], wp_sb[:, li, 1, :], Am_sb[:, 0:64],
                            op=OP.mult)
    fb = nwork.tile([128, 64], f32, tag="fb")
    nc.vector.tensor_tensor(fb[:], wp_sb[:, li, 2, :], inv[:], op=OP.mult)
    fc_ = nwork.tile([128, 64], f32, tag="fc_")
    nc.vector.tensor_tensor(fc_[:], fa[:], fb[:], op=OP.add)
    fw = nwork.tile([128, 64], f32, tag="fw")
    nc.vector.tensor_tensor(fw[:], fc_[:], wp_sb[:, li, 0, :], op=OP.add)
    B0 = nwork.tile([128, 64], f32, tag="B0")
    nc.vector.tensor_tensor(B0[:], Am_sb[:, 0:64], fw[:], op=OP.mult)

    if li == 0:
        ps_s = psS_p.tile([128, 256], f32, tag="ps_c")
        nc.tensor.matmul(ps_s[:, 0:64], lhsT=ohT_sb[:, t * 128:(t + 1) * 128],
                         rhs=sc0tab_sb[:], start=True, stop=True)
        fnew = feats0[:, t, :]
        nc.vector.tensor_tensor(fnew[:], B0[:], ps_s[:, 0:64], op=OP.add)
        mro = nwork.tile([128, 64], f32, tag="mro")
        nc.vector.tensor_tensor(mro[:], fnew[:], wro0_sb[:], op=OP.mult)
        nc.vector.reduce_sum(oute_sb[:, 0, t:t + 1], mro[:], axis=AX.X)
        nc.vector.tensor_copy(t1stage[:, t, :], fnew[:])
        ps_t = psS_p.tile([128, 256], f32, tag="ps_c")
        nc.tensor.transpose(ps_t[0:64, 0:128], fnew[:], identity=ident[:])
        nc.vector.tensor_copy(fT_all[:, t * 128:(t + 1) * 128],
                              ps_t[0:64, 0:128])
    else:
        fnew = nwork.tile([128, 64], f32, tag="fnew1")
        nc.vector.tensor_tensor(fnew[:], B0[:], sc1_sb[:, t, :], op=OP.add)
        ps_t = psS_p.tile([128, 256], f32, tag="ps_c")
        nc.tensor.transpose(ps_t[0:64, 0:128], fnew[:], identity=ident[:])
        fT = nwork.tile([64, 128], bf16, tag="fT")
        nc.vector.tensor_copy(fT[:], ps_t[0:64, 0:128])
        ps_h = psS_p.tile([128, 256], f32, tag="ps_c")
        nc.tensor.matmul(ps_h[:, 0:MLP_H], lhsT=fT[:], rhs=wm1_sb[:],
                         start=True, stop=True)
        hb = nwork.tile([128, MLP_H], f32, tag="hb")
        nc.vector.tensor_tensor(hb[:], ps_h[:, 0:MLP_H], bm1_sb[:], op=OP.add)
        hsg = nwork.tile([128, MLP_H], f32, tag="hsg")
        nc.scalar.activation(hsg[:], hb[:], AF.Silu)
        m2 = nwork.tile([128, MLP_H], f32, tag="m2")
        nc.vector.tensor_tensor(m2[:], hsg[:], wm2_sb[:], op=OP.mult)
        nc.vector.reduce_sum(oute_sb[:, 1, t:t + 1], m2[:], axis=AX.X)


def _host_prep(inputs):
    import heapq
    pos = np.asarray(inputs["positions"], np.float32)
    shifts = np.asarray(inputs["shifts"], np.float32)
    spec = np.asarray(inputs["species"]).astype(np.int64)
    snd = np.asarray(inputs["senders"]).astype(np.int64)
    rcv = np.asarray(inputs["receivers"]).astype(np.int64)
    W_embed = np.asarray(inputs["W_embed"], np.float32)
    W_rad = np.asarray(inputs["W_rad"], np.float32)
    W_mix = np.asarray(inputs["W_mix"], np.float32)
    W_prod = np.asarray(inputs["W_prod"], np.float32)
    W_sc = np.asarray(inputs["W_sc"], np.float32)
    W_ro0 = np.asarray(inputs["W_ro0"], np.float32)
    W_m1 = np.asarray(inputs["W_m1"], np.float32)
    b_m1 = np.asarray(inputs["b_m1"], np.float32)
    W_m2 = np.asarray(inputs["W_m2"], np.float32)

    NBIN = NCORE * NT
    deg = np.bincount(rcv, minlength=N)
    order = np.argsort(-deg, kind="stable")
    heap = [(0, 0, b) for b in range(NBIN)]
    heapq.heapify(heap)
    bin_nodes = [[] for _ in range(NBIN)]
    bin_load = np.zeros(NBIN, np.int64)
    for n_ in order:
        while True:
            load, cnt, b = heapq.heappop(heap)
            if cnt < 128:
                break
        bin_nodes[b].append(n_)
        bin_load[b] = load + deg[n_]
        heapq.heappush(heap, (int(bin_load[b]), cnt + 1, b))
    maxload = int(bin_load.max())
    ST = max(2, -(-maxload // 128))
    NSUB = NT * ST

    slot2node = np.empty((NCORE, NT, 128), np.int64)
    part_of = np.empty(N, np.int64)
    core_of = np.empty(N, np.int64)
    tile_of = np.empty(N, np.int64)
    for b in range(NBIN):
        c, t = b // NT, b % NT
        nodes = np.array(bin_nodes[b], np.int64)
        slot2node[c, t, :] = nodes
        part_of[nodes] = np.arange(128)
        core_of[nodes] = c
        tile_of[nodes] = t
    t1row = core_of * NPC + part_of * NT + tile_of

    ecore = core_of[rcv]
    etile = tile_of[rcv]

    es = np.zeros((NCORE, 128, NSUB, 3), np.float32)
    er = np.zeros((NCORE, 128, NSUB, 3), np.float32)
    sspec = -np.ones((NCORE, 128, NSUB), np.int64)
    idxf2 = np.zeros((NCORE, 128, NSUB), np.int32)
    recvb = -np.ones((NCORE, 128, NSUB), np.float32)

    for c in range(NCORE):
        in_c = np.nonzero(ecore == c)[0]
        t_c = etile[in_c]
        for t in range(NT):
            ee = in_c[t_c == t]
            cnt = len(ee)
            assert cnt <= ST * 128, f"tile overflow c{c} t{t}: {cnt}"
            sl = np.arange(cnt)
            p, col = sl % 128, t * ST + sl // 128
            es[c, p, col, :] = pos[snd[ee]]
            er[c, p, col, :] = pos[rcv[ee]] + shifts[ee]
            sspec[c, p, col] = spec[snd[ee]]
            idxf2[c, p, col] = t1row[snd[ee]]
            recvb[c, p, col] = part_of[rcv[ee]].astype(np.float32)

    bf = ml_dtypes.bfloat16
    soh = np.zeros((NCORE, 10, NSUB * 128), bf)
    pgrid, gsgrid = np.meshgrid(np.arange(128), np.arange(NSUB),
                                indexing="ij")
    for c in range(NCORE):
        sp = sspec[c]
        m = sp >= 0
        soh[c, sp[m], (gsgrid * 128 + pgrid)[m]] = 1

    ohT = np.zeros((NCORE, 10, NT * 128), bf)
    ohcols = np.zeros((NCORE, 128, NT * 10), np.float32)
    for c in range(NCORE):
        for t in range(NT):
            sp_t = spec[slot2node[c, t]]
            ohT[c, sp_t, t * 128 + np.arange(128)] = 1
            ohcols[c, np.arange(128), t * 10 + sp_t] = 1

    wrad_r = np.zeros((8, 2, 192), np.float32)
    for i in range(2):
        wrad_r[:, i, :] = W_rad[i].transpose(1, 0, 2).reshape(NB, 192)
    wmix_r = np.zeros((64, 2, 3, 64), np.float32)
    for i in range(2):
        for l in range(3):
            wmix_r[:, i, l, :] = W_mix[i, l] / AVG
    sc0tab = np.einsum("sk,skj->sj", W_embed, W_sc[0, :, 0])
    wall = np.ascontiguousarray(
        W_sc[1, :, 0].transpose(1, 0, 2).reshape(64, 640))
    wp_rep = np.zeros((128, 2, 3, 64), np.float32)
    for i in range(2):
        for j in range(3):
            wp_rep[:, i, j, :] = W_prod[i, j, 0][None, :]
    n_ = np.arange(1, NB + 1, dtype=np.float32)

    shared = dict(
        wrad_r=wrad_r.astype(bf),
        wmix_r=wmix_r.astype(bf),
        wemb_b=W_embed.astype(bf),
        sc0tab=sc0tab.astype(bf),
        wall=wall.astype(bf),
        wp_rep=wp_rep,
        wro0_rep=np.tile(W_ro0[None, :], (128, 1)).astype(np.float32),
        wm1_b=W_m1.astype(bf),
        bm1_rep=np.tile(b_m1[None, :], (128, 1)).astype(np.float32),
        wm2_rep=np.tile(W_m2[None, :], (128, 1)).astype(np.float32),
        npi_rep=np.tile((n_ * np.float32(PI))[None, :], (128, 1)),
        nh_rep=np.tile((n_ / 2.0)[None, :], (128, 1)).astype(np.float32),
    )
    in_maps = []
    for c in range(NCORE):
        m = dict(shared)
        m["es_d"] = np.ascontiguousarray(es[c].reshape(128, NSUB * 3))
        m["er_d"] = np.ascontiguousarray(er[c].reshape(128, NSUB * 3))
        m["soh_d"] = soh[c]
        m["recv_d"] = recvb[c].astype(bf)
        m["idxf2_d"] = idxf2[c]
        m["ohT"] = ohT[c]
        m["ohcols"] = ohcols[c]
        in_maps.append(m)
    return in_maps, slot2node, ST


def kernel(**inputs):
    from concourse import bass_utils
    in_maps, slot2node, ST = _host_prep(inputs)
    nc = _build_program(ST)
    res = bass_utils.run_bass_kernel_spmd(nc, in_maps,
                                          core_ids=list(range(NCORE)))
    out = np.zeros((N, 2), np.float32)
    for c in range(NCORE):
        oe = np.asarray(res.results[c]["out_e"], np.float32)
        for i in range(2):
            out[slot2node[c].reshape(-1), i] = oe[:, i, :].T.reshape(-1)
    return out
